# revision 29
# baseline (speedup 1.0000x reference)
"""ExclusiveSelfAttention TRN2 kernel: head-sharded tensor parallel over 8 NeuronCores.

Sharding: 16 heads / 8 cores = 2 heads (128 channels) per core.
Each core computes q/k/v projections for its 2 heads (full sequence),
attention + per-position Gram-Schmidt exclusion (head-local), and a
partial output projection (contraction over its 128 channels).
The host sums the 8 partials and adds the output bias.

All matmuls run in bf16 with fp32 PSUM accumulation. Attention is
computed transposed (scores^T[j, i]) so softmax-exp reads PSUM directly
on the ACT engine and the PV matmul needs no on-chip transposes of the
big tensors; sumexp rides along as a ones-column appended to v.
"""

import sys

if '/opt/trn_rl_repo' not in sys.path:
    sys.path.insert(0, '/opt/trn_rl_repo')

import numpy as np
import ml_dtypes

import concourse.bass as bass
import concourse.mybir as mybir
import concourse.tile as tile
from concourse.bass_utils import run_bass_kernel_spmd

F32 = mybir.dt.float32
BF16 = mybir.dt.bfloat16
AF = mybir.ActivationFunctionType
ALU = mybir.AluOpType

B, S, D = 2, 2048, 1024
BS = B * S                    # 4096 combined (b, s) rows
HD = 64                       # head dim
E_LOC = 128                   # channels per core (2 heads)
N_CORES = 8
EPS = 1e-8
INV_SQRT_HD = 0.125

_ENGINE_TO_NC = {"PE": "tensor", "DVE": "vector", "Activation": "scalar",
                 "Pool": "gpsimd", "SP": "sync"}


def _make_nop(nc, engine):
    eng = getattr(nc, _ENGINE_TO_NC[str(engine).split(".")[-1]])
    r = eng.nop(nofuse=True, hint="waitsplit")
    ins = r.ins if hasattr(r, "ins") else r
    for blk in nc.main_func.blocks:
        insns = blk.instructions
        for i, x in enumerate(insns):
            if x.name == ins.name:
                del insns[i]
                blk.instructions = insns
                return ins
    raise RuntimeError("freshly created nop not found")


def split_waits(nc, limit=1):
    """Walrus codegen only encodes one sync-wait per instruction here; move
    excess waits onto preceding same-engine NOPs (same-engine program order
    makes this semantics-preserving)."""
    for blk in nc.main_func.blocks:
        ins_list = blk.instructions
        out, changed = [], False
        for ins in ins_list:
            si = ins.sync_info
            if si is not None and len(si.on_wait) > limit:
                waits = list(si.on_wait)
                extra, keep = waits[:-limit], waits[-limit:]
                for w in extra:
                    nop = _make_nop(nc, ins.engine)
                    nop.sync_info = mybir.SyncInfo(on_wait=[w], on_update=[])
                    out.append(nop)
                ins.sync_info = mybir.SyncInfo(on_wait=keep, on_update=list(si.on_update))
                changed = True
            out.append(ins)
        if changed:
            blk.instructions = out


def build_program():
    nc = bass.Bass()

    xT_d = nc.declare_dram_parameter("xT", [D, BS], BF16, isOutput=False)
    wqT_d = nc.declare_dram_parameter("wqT", [D, E_LOC], BF16, isOutput=False)
    wkT_d = nc.declare_dram_parameter("wkT", [D, E_LOC], BF16, isOutput=False)
    wvT_d = nc.declare_dram_parameter("wvT", [D, E_LOC], BF16, isOutput=False)
    bq_d = nc.declare_dram_parameter("bq", [E_LOC], F32, isOutput=False)
    bk_d = nc.declare_dram_parameter("bk", [E_LOC], F32, isOutput=False)
    bv_d = nc.declare_dram_parameter("bv", [E_LOC], F32, isOutput=False)
    woT_d = nc.declare_dram_parameter("woT", [E_LOC, D], BF16, isOutput=False)
    part_d = nc.declare_dram_parameter("partial", [BS, D], F32, isOutput=True)

    with tile.TileContext(nc) as tc:
        import contextlib
        with contextlib.ExitStack() as ctx:
            const = ctx.enter_context(tc.tile_pool(name="const", bufs=1))
            xt_pool = ctx.enter_context(tc.tile_pool(name="xt", bufs=2))
            persist = ctx.enter_context(tc.tile_pool(name="persist", bufs=1))
            et_pool = ctx.enter_context(tc.tile_pool(name="et", bufs=39))
            vn_pool = ctx.enter_context(tc.tile_pool(name="vn", bufs=32))
            sb_x = ctx.enter_context(tc.tile_pool(name="sb_x", bufs=3))
            sb_s = ctx.enter_context(tc.tile_pool(name="sb_s", bufs=2))
            out_stage = ctx.enter_context(tc.tile_pool(name="ostg", bufs=4))
            dram = ctx.enter_context(tc.tile_pool(name="dram", bufs=1, space="DRAM"))
            ps_scA = ctx.enter_context(tc.tile_pool(name="ps_scA", bufs=1, space="PSUM"))
            ps_scB = ctx.enter_context(tc.tile_pool(name="ps_scB", bufs=1, space="PSUM"))
            ps_pv = ctx.enter_context(tc.tile_pool(name="ps_pv", bufs=2, space="PSUM"))
            ps_x = ctx.enter_context(tc.tile_pool(name="ps_x", bufs=2, space="PSUM"))

            # ---- constants / weights ----
            wsb = {}
            for name, wd in (("q", wqT_d), ("k", wkT_d), ("v", wvT_d)):
                t = const.tile([128, 8, E_LOC], BF16, tag=f"w{name}")
                for kt in range(8):
                    nc.sync.dma_start(out=t[:, kt, :], in_=wd[kt * 128:(kt + 1) * 128, :])
                wsb[name] = t
            bsb = {}
            for name, bd in (("q", bq_d), ("k", bk_d)):
                t = const.tile([128, 1], F32, tag=f"b{name}")
                nc.sync.dma_start(out=t, in_=bd[:].rearrange("(p one) -> p one", one=1))
                bsb[name] = t
            # v bias split per head so both halves live at partition base 0
            bv_h = []
            for h in range(2):
                t = const.tile([64, 1], F32, tag=f"bv{h}")
                nc.sync.dma_start(out=t, in_=bv_d[h * 64:(h + 1) * 64]
                                  .rearrange("(p one) -> p one", one=1))
                bv_h.append(t)
            wo_sb = const.tile([128, D], BF16, tag="wo")
            nc.sync.dma_start(out=wo_sb, in_=woT_d[:, :])

            ones64 = const.tile([64, 1], BF16, tag="ones64")
            nc.vector.memset(ones64, 1.0)
            ones_row = const.tile([128, 32], BF16, tag="ones_row")
            nc.vector.memset(ones_row, 1.0)
            # K=1 broadcast matmul weight: [1, 64] ones
            ones1 = const.tile([1, 64], BF16, tag="ones1")
            nc.vector.memset(ones1, 1.0)

            # ---- persistent activations ----
            qT = persist.tile([128, BS], BF16, tag="qT")       # [e_loc, b*s]
            kT = persist.tile([128, BS], BF16, tag="kT")
            vT = persist.tile([64, 2 * BS], BF16, tag="vT")     # head-major: [:, h*BS + s]
            o_fT = {(b, ih): persist.tile([128, 1024], BF16, tag=f"ofT{b}{ih}",
                                          name=f"ofT{b}{ih}")
                    for b in range(B) for ih in range(2)}

            # ---- phase 1: projections ----
            for sb8 in range(8):
                scols = slice(sb8 * 512, (sb8 + 1) * 512)
                xt = xt_pool.tile([128, 8, 512], BF16, tag="xt")
                for kt in range(8):
                    nc.sync.dma_start(out=xt[:, kt, :],
                                      in_=xT_d[kt * 128:(kt + 1) * 128, scols])
                for name in ("q", "k", "v"):
                    psp = ps_x.tile([128, 512], F32, tag="ps_x")
                    for kt in range(8):
                        nc.tensor.matmul(psp, wsb[name][:, kt, :], xt[:, kt, :],
                                         start=(kt == 0), stop=(kt == 7))
                    if name == "q":
                        nc.scalar.activation(qT[:, scols], psp, AF.Identity,
                                             bias=bsb[name], scale=1.0)
                    elif name == "k":
                        nc.scalar.activation(kT[:, scols], psp, AF.Identity,
                                             bias=bsb[name], scale=1.0)
                    else:
                        nc.scalar.activation(vT[:, sb8 * 512:(sb8 + 1) * 512],
                                             psp[0:64, :], AF.Identity,
                                             bias=bv_h[0], scale=1.0)
                        vtmp = sb_x.tile([64, 512], F32, tag="vtmp")
                        nc.vector.tensor_copy(vtmp, psp[64:128, :])
                        nc.vector.tensor_scalar(out=vT[:, BS + sb8 * 512:BS + (sb8 + 1) * 512],
                                                in0=vtmp,
                                                scalar1=bv_h[1], scalar2=None, op0=ALU.add)

            # ---- phase 1.5: v natural via DRAM round-trip with DMA transpose ----
            # vdram rows: 0:64 head A, 64 ones, 65:129 head B, 129 ones, 130:144 pad
            from concourse.tile import add_dep_helper
            vdram = dram.tile([144, BS], BF16, tag="vdram")

            def _row_ap(r):
                return vdram[r:r + 1, :].rearrange("one (p f) -> (one p) f", p=128)

            vdw_const = [nc.gpsimd.dma_start(out=_row_ap(64), in_=ones_row),
                         nc.gpsimd.dma_start(out=_row_ap(129), in_=ones_row)]
            vdw_const += [nc.gpsimd.dma_start(out=_row_ap(130 + pr), in_=ones_row)
                          for pr in range(14)]
            vdw_b = []
            for b in range(B):
                bc = slice(b * S, (b + 1) * S)
                vdw_b.append([
                    nc.gpsimd.dma_start(out=vdram[0:64, bc], in_=vT[:, bc]),
                    nc.gpsimd.dma_start(out=vdram[65:129, bc],
                                        in_=vT[:, BS + b * S:BS + (b + 1) * S]),
                ])
            vn = []
            for jt in range(32):          # global j-tile over b*s
                t = vn_pool.tile([128, 144], BF16, tag="vn")
                rd = nc.sync.dma_start(out=t, in_=vdram[:, jt * 128:(jt + 1) * 128],
                                       transpose=True)
                for w in vdw_const + vdw_b[jt // 16]:
                    add_dep_helper(rd.ins if hasattr(rd, "ins") else rd,
                                   w.ins if hasattr(w, "ins") else w,
                                   reason="vdram write before transpose read")
                vn.append(t)

            # ---- phase 2: attention + exclusion + out-proj ----
            for b in range(B):
                for ih in range(2):                       # i-halves of 1024
                    i0 = b * S + ih * 1024                # global i offset in [0, BS)
                    et = {}
                    with tc.high_priority(offset=250):
                        for jt in range(16):
                            jcol = slice(b * S + jt * 128, b * S + (jt + 1) * 128)
                            psA = ps_scA.tile([128, 1024], F32, tag="scA")
                            psB = ps_scB.tile([128, 1024], F32, tag="scB")
                            for h, (pst, tp) in ((0, (psA, (0, 0))), (1, (psB, (64, 0)))):
                                hp = slice(h * 64, (h + 1) * 64)
                                for s2 in range(2):
                                    icols = slice(i0 + s2 * 512, i0 + (s2 + 1) * 512)
                                    nc.tensor.matmul(pst[:, s2 * 512:(s2 + 1) * 512],
                                                     kT[hp, jcol], qT[hp, icols],
                                                     start=True, stop=True, tile_position=tp)
                            for h, pst in ((0, psA), (1, psB)):
                                e_t = et_pool.tile([128, 1024], BF16, tag="et")
                                nc.scalar.activation(e_t, pst, AF.Exp, bias=0.0,
                                                     scale=INV_SQRT_HD)
                                et[(h, jt)] = e_t

                    for h in range(2):
                        # vv = sum_c v^2 per position, for this (b, h, ih) i-range
                        vcols = slice(h * BS + b * S + ih * 1024,
                                      h * BS + b * S + (ih + 1) * 1024)
                        tvv = sb_x.tile([64, 1024], BF16, tag="tvv")
                        nc.vector.tensor_tensor(out=tvv, in0=vT[:, vcols],
                                                in1=vT[:, vcols], op=ALU.mult)
                        vrec = sb_s.tile([1, 1024], F32, tag="vrec")
                        for s2 in range(2):
                            ps_vv = ps_x.tile([1, 512], F32, tag="ps_x")
                            nc.tensor.matmul(ps_vv, ones64, tvv[:, s2 * 512:(s2 + 1) * 512],
                                             start=True, stop=True)
                            veps = sb_s.tile([1, 512], F32, tag="veps")
                            nc.vector.tensor_scalar(out=veps, in0=ps_vv, scalar1=EPS,
                                                    scalar2=None, op0=ALU.add)
                            nc.vector.reciprocal(vrec[:, s2 * 512:(s2 + 1) * 512], veps)

                        for i2 in range(2):
                            pso = ps_pv.tile([65, 512], F32, tag="pv",
                                             name=f"pv{b}{ih}{h}{i2}")
                            for jt in range(16):
                                vt_jt = vn[b * 16 + jt]
                                nc.tensor.matmul(pso, vt_jt[:, h * 65:h * 65 + 65],
                                                 et[(h, jt)][:, i2 * 512:(i2 + 1) * 512],
                                                 start=(jt == 0), stop=(jt == 15))
                            ib_cols = slice(i2 * 512, (i2 + 1) * 512)
                            vcols2 = slice(h * BS + i0 + i2 * 512, h * BS + i0 + (i2 + 1) * 512)
                            # exclusion: o_f = (o~ - align*v) * r
                            tov = sb_x.tile([64, 512], BF16, tag="tov")
                            nc.vector.tensor_tensor(out=tov, in0=pso[0:64, :],
                                                    in1=vT[:, vcols2], op=ALU.mult)
                            ps_ov = ps_x.tile([1, 512], F32, tag="ps_x")
                            nc.tensor.matmul(ps_ov, ones64, tov,
                                             start=True, stop=True)
                            r_t = sb_s.tile([1, 512], BF16, tag="r_t", bufs=4)
                            with nc.allow_low_precision(reason="softmax scale in bf16 by design"):
                                nc.vector.reciprocal(r_t, pso[64:65, :])
                            align = sb_s.tile([1, 512], BF16, tag="align", bufs=4)
                            nc.vector.tensor_tensor(out=align, in0=ps_ov,
                                                    in1=vrec[:, ib_cols], op=ALU.mult)
                            ps_bc = ps_x.tile([128, 512], F32, tag="ps_x")
                            nc.tensor.matmul(ps_bc[0:64, :], ones1, r_t,
                                             start=True, stop=True, tile_position=(0, 0))
                            nc.tensor.matmul(ps_bc[64:128, :], ones1, align,
                                             start=True, stop=True, tile_position=(0, 64))
                            t2 = sb_x.tile([64, 512], F32, tag="t2")
                            nc.vector.tensor_tensor(out=t2, in0=ps_bc[64:128, :],
                                                    in1=vT[:, vcols2], op=ALU.mult)
                            t3 = sb_x.tile([64, 512], F32, tag="t3")
                            nc.vector.tensor_tensor(out=t3, in0=pso[0:64, :],
                                                    in1=t2, op=ALU.subtract)
                            nc.vector.tensor_tensor(
                                out=o_fT[(b, ih)][h * 64:(h + 1) * 64,
                                                  i2 * 512:(i2 + 1) * 512],
                                in0=ps_bc[0:64, :], in1=t3, op=ALU.mult)

                    # out projection for this i-half: overlaps with the next
                    # round's attention instead of forming a serial tail.
                    for st8 in range(8):
                        st = 8 * ih + st8
                        for eb in range(2):
                            ps_o2 = ps_x.tile([128, 512], F32, tag="ps_x")
                            nc.tensor.matmul(ps_o2,
                                             o_fT[(b, ih)][:, st8 * 128:(st8 + 1) * 128],
                                             wo_sb[:, eb * 512:(eb + 1) * 512],
                                             start=True, stop=True)
                            stg = out_stage.tile([128, 512], F32, tag="ostg")
                            if b == 1 and ih == 1:
                                nc.scalar.copy(stg, ps_o2)
                            else:
                                nc.vector.tensor_copy(stg, ps_o2)
                            nc.sync.dma_start(
                                out=part_d[b * S + st * 128:b * S + (st + 1) * 128,
                                           eb * 512:(eb + 1) * 512],
                                in_=stg)

    split_waits(nc)
    return nc


_CACHE = {}


def kernel(x, wq, bq, wk, bk, wv, bv, wo, bo):
    x = np.ascontiguousarray(np.asarray(x, dtype=np.float32))
    wq, wk, wv, wo = (np.asarray(w, dtype=np.float32) for w in (wq, wk, wv, wo))
    bq, bk, bv, bo = (np.asarray(v, dtype=np.float32) for v in (bq, bk, bv, bo))

    if "nc" not in _CACHE:
        _CACHE["nc"] = build_program()
    nc = _CACHE["nc"]

    xT = np.ascontiguousarray(x.reshape(BS, D).T).astype(ml_dtypes.bfloat16)
    in_maps = []
    for g in range(N_CORES):
        cs = slice(g * E_LOC, (g + 1) * E_LOC)
        in_maps.append({
            "xT": xT,
            "wqT": np.ascontiguousarray(wq[cs, :].T).astype(ml_dtypes.bfloat16),
            "wkT": np.ascontiguousarray(wk[cs, :].T).astype(ml_dtypes.bfloat16),
            "wvT": np.ascontiguousarray(wv[cs, :].T).astype(ml_dtypes.bfloat16),
            "bq": np.ascontiguousarray(bq[cs]),
            "bk": np.ascontiguousarray(bk[cs]),
            "bv": np.ascontiguousarray(bv[cs]),
            "woT": np.ascontiguousarray(wo[:, cs].T).astype(ml_dtypes.bfloat16),
        })

    res = run_bass_kernel_spmd(nc, in_maps, list(range(N_CORES)))
    out = np.zeros((BS, D), np.float32)
    for g in range(N_CORES):
        out += np.asarray(res.results[g]["partial"], np.float32)
    out += bo[None, :]
    return out.reshape(B, S, D)
